# revision 22
# baseline (speedup 1.0000x reference)
"""Trainium2 Bass kernel for the RNN-T style Joint network:

    out[b,t,u,v] = sum_k tanh(enc_p[b,t,k] + dec_p[b,u,k] + b1[k]) * W2[v,k] + b2[v]
    enc_p = h_enc @ W1[:, :H].T ; dec_p = h_dec @ W1[:, H:].T

Sharding: data-parallel over B across 8 NeuronCores (B == 8, one batch row per
core). Weights are replicated. No collectives needed.

Per-core pipeline (one NeuronCore):
  warmup: ~36 dummy N=128 matmuls on a zeroed tile fill the initial DMA-wait
      window so the PE HAM clock-gate reaches 2.4 GHz before GEMM1.
  inputs: ONE batched DMA per tensor (6 total), spread over the SP and
      gpsimd queues; the compute engines' FIFOs stay free of DMA issue.
  GEMM1 (PE, bf16): enc_pT [HID, T] and dec_pT [HID, U] in transposed layout
      (HID on partitions); b1 folded in via the ScalarE per-partition bias
      during PSUM->SBUF evacuation.
  broadcast-add (VectorE) + tanh (ScalarE): hT [HID, tu-chunk] bf16 via
      stride-0 broadcast access patterns. Chunks 0-1 are built in 512-wide
      halves so the first GEMM2 block starts ~5us earlier; later chunks are
      built 1024-wide, two chunks ahead of consumption.
  GEMM2 (PE, bf16), flipped: outT[v, tu] = w2T_blk.T @ hT accumulated over
      5 K-tiles in fp32 PSUM. Per 512-wide tu-block, 8 psum banks hold the 8
      v-blocks of 128; v0-outer / kk-inner order gives each bank ~35 matmuls
      of runway between its last write and its next-block reuse.
  evac (split): psum[v,tu] + b2 (per-partition bias) -> bf16 into a per-
      parity [128, 2048] staging tile. Even v-blocks on ScalarE (activation
      Identity with bias), odd on VectorE (tensor_add with stride-0 b2).
  DMA out: ONE 512 KB DMA per parity per block (64 total) on the SP queue,
      4 v-blocks per DMA via a 3-D access pattern. Output is bf16 [V, TU];
      the host transposes and upcasts (adds ~0.1% rel err, halves HBM
      writes, and the small DMA/semaphore count shortens the postamble).
"""

import numpy as np
import ml_dtypes

B, T, U, H = 8, 256, 64, 512
HID, V = 640, 1024
TU = T * U  # 16384
N_CORES = 8
N_CHUNKS = TU // 1024  # 16 chunks of 16 t-values x 64 u-values
N_BLOCKS = TU // 512  # 32 GEMM2 tu-blocks
KK = HID // 128  # 5 K-tiles

BF16 = ml_dtypes.bfloat16

_CACHE = {}


def _build_bass():
    import concourse.bass as bass
    import concourse.tile as tile
    from concourse import bacc, mybir

    f32 = mybir.dt.float32
    bf16 = mybir.dt.bfloat16
    Tanh = mybir.ActivationFunctionType.Tanh

    nc = bacc.Bacc("TRN2", target_bir_lowering=False, debug=False,
                   num_devices=N_CORES)

    # inputs arrive pre-arranged in the exact SBUF layout (partition-major,
    # k-tiles concatenated along the free dim) and pre-merged by need-time,
    # so the whole load is THREE contiguous 2-D DMAs (DMA fixed cost ~0.8us
    # each dominates for small transfers):
    #   encIn = w1-enc-half ++ henc   (GEMM1-enc critical)
    #   decIn = w1-dec-half ++ hdec
    #   w2P   = w2
    ENC_F = 4 * HID + 4 * T
    DEC_F = 4 * HID + 4 * U
    g1P = nc.dram_tensor("g1P", [128, ENC_F + DEC_F], bf16,
                         kind="ExternalInput").ap()
    w2P = nc.dram_tensor("w2P", [128, KK * V], bf16, kind="ExternalInput").ap()
    b1c5 = nc.dram_tensor("b1c5", [128, KK], f32, kind="ExternalInput").ap()
    b2c8 = nc.dram_tensor("b2c8", [128, 8], f32, kind="ExternalInput").ap()
    outT = nc.dram_tensor("outT", [V, TU], bf16, kind="ExternalOutput").ap()

    def bcast3(ap2d, mid):
        """[P, N] AP -> [P, mid, N] with a stride-0 middle dim."""
        return bass.AP(tensor=ap2d.tensor, offset=ap2d.offset,
                       ap=[ap2d.ap[0], [0, mid], ap2d.ap[1]])

    def repeat3(ap2d, inner):
        """[P, N] AP -> [P, N, inner] with a stride-0 inner dim."""
        return bass.AP(tensor=ap2d.tensor, offset=ap2d.offset,
                       ap=[ap2d.ap[0], ap2d.ap[1], [0, inner]])

    def bcast_free(ap2d, n):
        """[P, 1] AP -> [P, n] with a stride-0 free dim."""
        return bass.AP(tensor=ap2d.tensor, offset=ap2d.offset,
                       ap=[ap2d.ap[0], [0, n]])

    def grouped3(ap2d, gstride, g, inner):
        """[P, >=g*inner] AP -> [P, g, inner] with group stride gstride."""
        return bass.AP(tensor=ap2d.tensor, offset=ap2d.offset,
                       ap=[ap2d.ap[0], [gstride, g], [1, inner]])

    with tile.TileContext(nc) as tc:
        with (
            tc.tile_pool(name="consts", bufs=1) as consts,
            tc.tile_pool(name="psum", bufs=1, space="PSUM") as psum,
            tc.tile_pool(name="prep", bufs=4) as prep,
            tc.tile_pool(name="hTp", bufs=3) as hTp,
            tc.tile_pool(name="outp", bufs=6) as outp,
        ):
            # ---- PE warmup: keep the HAM activity window busy during the
            # initial input-DMA wait so GEMM1 runs at 2.4 GHz. zt is zeroed
            # by gpsimd (ready ~6us, right after the NEFF preamble). ----
            zt = consts.tile([128, 128], bf16, tag="z", name="z")
            nc.gpsimd.memset(zt[:, :], 0)
            psw = psum.tile([128, 512], f32, tag="mm", bufs=8, name="warm")
            for _ in range(54):
                nc.tensor.matmul(psw[:, :128], lhsT=zt[:, :], rhs=zt[:, :],
                                 start=True, stop=True)

            # ---- input DMAs: ONE merged load for all GEMM1 inputs (both
            # halves are needed near-simultaneously; one DMA pays the fixed
            # cost once), then w2, serially on the SP queue. Tiny biases
            # ride the idle gpsimd queue.
            g1B = consts.tile([128, ENC_F + DEC_F], bf16, tag="g1B",
                              name="g1B")
            nc.sync.dma_start(out=g1B, in_=g1P[:, :])
            w2B = consts.tile([128, KK * V], bf16, tag="w2B", name="w2B")
            nc.sync.dma_start(out=w2B, in_=w2P[:, :])

            b1_t = consts.tile([128, KK], f32, tag="b1", name="b1")
            nc.gpsimd.dma_start(out=b1_t, in_=b1c5[:, :])
            b2_t = consts.tile([128, 8], f32, tag="b2", name="b2")
            nc.gpsimd.dma_start(out=b2_t, in_=b2c8[:, :])

            encB = g1B[:, :ENC_F]
            decB = g1B[:, ENC_F:ENC_F + DEC_F]
            henc_t = [encB[:, 4 * HID + k * T: 4 * HID + (k + 1) * T]
                      for k in range(4)]
            hdec_t = [decB[:, 4 * HID + k * U: 4 * HID + (k + 1) * U]
                      for k in range(4)]
            w1_t = ([encB[:, k * HID:(k + 1) * HID] for k in range(4)]
                    + [decB[:, k * HID:(k + 1) * HID] for k in range(4)])
            w2_t = [w2B[:, k * V:(k + 1) * V] for k in range(KK)]

            # ---- GEMM1 (bf16): enc_pT [HID, T], dec_pT [HID, U] ----
            encbT = []
            decT = []
            for kk in range(KK):
                ps = psum.tile([128, 512], f32, tag="mm", bufs=8,
                               name=f"pse{kk}")
                for k in range(4):
                    nc.tensor.matmul(
                        ps[:, :T],
                        lhsT=w1_t[k][:, kk * 128:(kk + 1) * 128],
                        rhs=henc_t[k],
                        start=(k == 0), stop=(k == 3),
                    )
                e_ = consts.tile([128, T], f32, tag=f"encbT{kk}", name=f"encbT{kk}")
                # encbT = enc_pT + b1 (per-partition bias)
                nc.scalar.add(out=e_, in_=ps[:, :T], add=b1_t[:, kk:kk + 1])
                encbT.append(e_)
                psd = psum.tile([128, 512], f32, tag="mm", bufs=8,
                                name=f"psd{kk}")
                for k in range(4):
                    nc.tensor.matmul(
                        psd[:, :U],
                        lhsT=w1_t[4 + k][:, kk * 128:(kk + 1) * 128],
                        rhs=hdec_t[k],
                        start=(k == 0), stop=(k == 3),
                    )
                d_ = consts.tile([128, U], f32, tag=f"decT{kk}", name=f"decT{kk}")
                nc.scalar.copy(out=d_, in_=psd[:, :U])
                decT.append(d_)

            # ---- hT production ----
            # hblk[blk] = list over kk of (tile, column offset)
            hblk = {}

            def emit_build_half(c, half):
                """512-wide build (8 t-values) -- startup latency path."""
                hts = []
                for kk in range(KK):
                    pre = prep.tile([128, 512], f32, tag=f"preh{kk}",
                                    name=f"preh{c}_{half}_{kk}", bufs=2)
                    pre_ap = pre[:, :]
                    out3 = bass.AP(tensor=pre_ap.tensor, offset=pre_ap.offset,
                                   ap=[pre_ap.ap[0], [64, 8], [1, 64]])
                    t0 = c * 16 + half * 8
                    nc.vector.tensor_add(
                        out=out3,
                        in0=bcast3(decT[kk][:, :], 8),
                        in1=repeat3(encbT[kk][:, t0:t0 + 8], 64),
                    )
                    ht = hTp.tile([128, 512], bf16, tag=f"hTh{kk}",
                                  name=f"hTh{c}_{half}_{kk}", bufs=2)
                    nc.scalar.activation(out=ht, in_=pre, func=Tanh)
                    hts.append((ht, 0))
                hblk[2 * c + half] = hts

            def emit_build(c):
                """1024-wide build (16 t-values) -- steady state."""
                hts = []
                for kk in range(KK):
                    pre = prep.tile([128, 1024], f32, tag=f"pre{kk}",
                                    name=f"pre{c}_{kk}", bufs=2)
                    pre_ap = pre[:, :]
                    out3 = bass.AP(tensor=pre_ap.tensor, offset=pre_ap.offset,
                                   ap=[pre_ap.ap[0], [64, 16], [1, 64]])
                    nc.vector.tensor_add(
                        out=out3,
                        in0=bcast3(decT[kk][:, :], 16),
                        in1=repeat3(encbT[kk][:, c * 16:(c + 1) * 16], 64),
                    )
                    ht = hTp.tile([128, 1024], bf16, tag=f"hT{kk}",
                                  name=f"hT{c}_{kk}", bufs=3)
                    nc.scalar.activation(out=ht, in_=pre, func=Tanh)
                    hts.append(ht)
                hblk[2 * c] = [(ht, 0) for ht in hts]
                hblk[2 * c + 1] = [(ht, 512) for ht in hts]

            emit_build_half(0, 0)
            emit_build_half(0, 1)

            # ---- main GEMM2 loop.  Builds are emitted at the END of each
            # block so PSUM evacuations sit AHEAD of the (slack-rich)
            # broadcast-adds / tanhs in the in-order DVE/ScalarE queues. ----
            for blk in range(N_BLOCKS):
                hts = hblk[blk]
                tu0 = blk * 512
                ot = [None, None]
                for par in range(2):
                    ot[par] = outp.tile([128, 2048], bf16, tag=f"o{par}",
                                        bufs=3, name=f"ot{blk}_{par}")
                for v0 in range(8):
                    ps = psum.tile([128, 512], f32, tag="mm", bufs=8,
                                   name=f"ps{blk}_{v0}")
                    for kk in range(KK):
                        ht, off = hts[kk]
                        nc.tensor.matmul(
                            ps,
                            lhsT=w2_t[kk][:, v0 * 128:(v0 + 1) * 128],
                            rhs=ht[:, off:off + 512],
                            start=(kk == 0), stop=(kk == KK - 1),
                        )
                    par, j = v0 % 2, v0 // 2
                    dst_sl = ot[par][:, j * 512:(j + 1) * 512]
                    if par == 0:
                        nc.scalar.add(out=dst_sl, in_=ps,
                                      add=b2_t[:, v0:v0 + 1])
                    else:
                        nc.vector.tensor_add(
                            out=dst_sl, in0=ps,
                            in1=bcast_free(b2_t[:, v0:v0 + 1], 512))
                # one 512KB DMA per parity (4 v-blocks via 3-D dst pattern),
                # on the queue of the engine that produced the tile. The
                # final block goes out as 256KB halves so the run doesn't
                # end waiting on one long transfer.
                nsplit = 2 if blk >= N_BLOCKS - 2 else 1
                for par in range(2):
                    for s in range(nsplit):
                        g = 4 // nsplit
                        dst = bass.AP(tensor=outT.tensor,
                                      offset=(par + 2 * s * g) * 128 * TU + tu0,
                                      ap=[[TU, 128], [256 * TU, g], [1, 512]])
                        eng = nc.scalar if par == 0 else nc.sync
                        eng.dma_start(
                            out=dst,
                            in_=grouped3(ot[par][:, s * g * 512:], 512, g, 512))
                del hblk[blk]
                # trailing build for a later block (build(c) lands at the end
                # of blk 2c-3, two blocks before its first consumer blk 2c)
                if blk == 0:
                    emit_build_half(1, 0)
                    emit_build_half(1, 1)
                elif blk % 2 == 1:
                    c_next = (blk - 1) // 2 + 2
                    if c_next < N_CHUNKS:
                        emit_build(c_next)

    nc.finalize()
    return nc


def _get_nc():
    if "nc" not in _CACHE:
        _CACHE["nc"] = _build_bass()
    return _CACHE["nc"]


def _make_in_maps(h_enc, h_dec, W1, b1, W2, b2):
    h_enc = np.asarray(h_enc, dtype=np.float32)
    h_dec = np.asarray(h_dec, dtype=np.float32)
    W1 = np.asarray(W1, dtype=np.float32)
    b1 = np.asarray(b1, dtype=np.float32)
    W2 = np.asarray(W2, dtype=np.float32)
    b2 = np.asarray(b2, dtype=np.float32)

    def part_major(xT, nk):
        """[nk*128, F] -> [128, nk*F]: partition p holds rows p, p+128, ..."""
        f = xT.shape[1]
        return np.ascontiguousarray(
            xT.reshape(nk, 128, f).transpose(1, 0, 2).reshape(128, nk * f))

    w1T = W1.T.astype(BF16)                             # [2H, HID] bf16
    w1aP = part_major(w1T[:4 * 128], 4)
    w1bP = part_major(w1T[4 * 128:], 4)
    w2P = part_major(W2.T.astype(BF16), KK)             # [128, 5*V]
    b1c5 = np.ascontiguousarray(b1.reshape(KK, 128).T)  # [128, 5] f32
    b2c8 = np.ascontiguousarray(b2.reshape(8, 128).T)   # [128, 8] f32

    in_maps = []
    for b in range(N_CORES):
        g1P = np.concatenate(
            [w1aP, part_major(h_enc[b].T.astype(BF16), 4),
             w1bP, part_major(h_dec[b].T.astype(BF16), 4)], axis=1)
        in_maps.append({
            "g1P": np.ascontiguousarray(g1P),   # [128, 8*HID+4T+4U]
            "w2P": w2P,
            "b1c5": b1c5,
            "b2c8": b2c8,
        })
    return in_maps


def _run(in_maps, **kwargs):
    from concourse import bass_utils
    nc = _get_nc()
    return bass_utils.run_bass_kernel_spmd(
        nc, in_maps, core_ids=list(range(N_CORES)), **kwargs)


def kernel(h_enc, h_dec, W1, b1, W2, b2):
    in_maps = _make_in_maps(h_enc, h_dec, W1, b1, W2, b2)
    res = _run(in_maps)
    outs = [np.asarray(r["outT"]).T.astype(np.float32).reshape(T, U, V)
            for r in res.results]
    return np.stack(outs, axis=0)
